# revision 9
# baseline (speedup 1.0000x reference)
"""Trainium2 Bass kernel for nn_CategoryMultiplier.

out[b, s, :] = inputs[b, s, :] * (emb_table[categories[b, s]] if
               categories[b, s] != 0 else 1.0)

Sharding: pure data parallel over batch. 8 cores x 16 batches each.

Precision: fp16 end-to-end (host converts f32->fp16 in, fp16->f32 out).
The kernel is DMA-bus bound (~360 GB/s/core shared across all traffic),
so halving bytes moved halves the roofline. fp16 RNE keeps rel err
~7e-4, far under the 2e-2 gate.

Gather: GPSIMD ap_gather (pure SBUF compute on the Q7 cores) instead of
dma_gather. A dma_gather serializes ~8.4ns/row through the single SWDGE
queue AND burns 8.4 MB of DMA-bus traffic; ap_gather does neither. The
embedding table lives SBUF-resident in feature-split layout
[128 partitions, 1000 vocab, 4 feats] (8KB/partition), and each
16-partition Q7 group gathers its partitions' 4-feature slivers for
every position. DMA then only moves x in and y out (16.8 MB total).

Layouts (host-prepped, pure transposes):
  xt[p, n, f]     = x[n, 4p+f]          [128, N*4] f16
  tablet[p, v, f] = table[v, 4p+f]      [128, 4000] f16 (row 0 := 1.0)
  cats16[p, s]    = cats[16s + p%16]    [128, N/16] i16 (wrap 16, tile 8)
ap_gather per chunk: g[p, i, :] = tablet[p, cats[lo+i], :]; DVE mul
against xt chunk; result DMA'd back in the same transposed layout and
un-transposed on the host.

Padding (category 0 -> multiplier 1.0): host writes ones into table row
0 before transposing; index 0 then gathers 1.0s.
"""

import numpy as np

import concourse.bass as bass
import concourse.bacc as bacc
import concourse.mybir as mybir
import concourse.tile as tile
from concourse.bass_utils import run_bass_kernel_spmd

# Problem shape (hardcoded per harness contract).
B, S, D = 128, 512, 512
VOCAB = 1000
N_CORES = 8
B_LOC = B // N_CORES            # 16 batches per core
N = B_LOC * S                   # 8192 positions per core
P = 128                         # SBUF partitions
DSUB = D // P                   # 4 features per partition

F16 = mybir.dt.float16
I16 = mybir.dt.int16

# Position chunks: small head/tail to prime/drain the pipeline.
CHUNKS = [256, 768] + [1024] * 6 + [768, 256]
assert sum(CHUNKS) == N
T_CH = max(CHUNKS)


def _build_nc():
    nc = bacc.Bacc("TRN2", target_bir_lowering=False, debug=False)

    xt = nc.dram_tensor("xt", [P, N * DSUB], F16, kind="ExternalInput")
    cats16 = nc.dram_tensor("cats16", [P, N // 16], I16, kind="ExternalInput")
    tablet = nc.dram_tensor("tablet", [P, VOCAB * DSUB], F16,
                            kind="ExternalInput")
    yt = nc.dram_tensor("yt", [P, N * DSUB], F16, kind="ExternalOutput")

    # Issue the GPSIMD ucode library load BEFORE the TileContext so the
    # IRAM load overlaps Tile's own prologue barrier.
    from concourse.library_config import ap_gather as ap_gather_lib
    nc.gpsimd.load_library(ap_gather_lib)

    with tile.TileContext(nc) as tc:
        with (
            tc.tile_pool(name="const", bufs=1) as const_pool,
            tc.tile_pool(name="io", bufs=5) as io_pool,
            tc.tile_pool(name="gat", bufs=6) as gat_pool,
        ):
            cats_t = const_pool.tile([P, N // 16], I16)
            nc.scalar.dma_start(out=cats_t[:], in_=cats16[:])
            table_t = const_pool.tile([P, VOCAB * DSUB], F16)
            nc.scalar.dma_start(out=table_t[:], in_=tablet[:])
            table_ap = table_t[:].rearrange("p (v f) -> p v f", f=DSUB)

            pos = 0
            for tch in CHUNKS:
                lo, hi = pos * DSUB, (pos + tch) * DSUB
                g_t = gat_pool.tile([P, T_CH * DSUB], F16, tag="g")
                nc.gpsimd.ap_gather(
                    out_ap=g_t[:, :tch * DSUB].rearrange(
                        "p (n f) -> p n f", f=DSUB),
                    in_ap=table_ap,
                    idxs_ap=cats_t[:, pos // 16:(pos + tch) // 16],
                    channels=P,
                    num_elems=VOCAB,
                    d=DSUB,
                    num_idxs=tch,
                )

                x_t = io_pool.tile([P, T_CH * DSUB], F16, tag="x")
                nc.sync.dma_start(out=x_t[:, :tch * DSUB], in_=xt[:, lo:hi])

                nc.vector.tensor_mul(out=g_t[:, :tch * DSUB],
                                     in0=g_t[:, :tch * DSUB],
                                     in1=x_t[:, :tch * DSUB])
                nc.scalar.dma_start(out=yt[:, lo:hi], in_=g_t[:, :tch * DSUB])
                pos += tch

    nc.compile()
    return nc


_NC = None


def _get_nc():
    global _NC
    if _NC is None:
        _NC = _build_nc()
    return _NC


def _shard_inputs(inputs, categories, emb_table):
    tab = np.asarray(emb_table).astype(np.float16)
    tab[0, :] = np.float16(1.0)
    tab_tr = np.ascontiguousarray(
        tab.reshape(VOCAB, P, DSUB).transpose(1, 0, 2)
    ).reshape(P, VOCAB * DSUB)
    in_maps = []
    for i in range(N_CORES):
        xs = np.asarray(
            inputs[i * B_LOC:(i + 1) * B_LOC]
        ).astype(np.float16).reshape(N, P, DSUB)
        xs = np.ascontiguousarray(xs.transpose(1, 0, 2)).reshape(P, N * DSUB)
        c = categories[i * B_LOC:(i + 1) * B_LOC].reshape(N).astype(np.int16)
        cw = np.ascontiguousarray(np.tile(c.reshape(N // 16, 16).T, (8, 1)))
        in_maps.append({"xt": xs, "cats16": cw, "tablet": tab_tr})
    return in_maps


def kernel(inputs, categories, mask_positions=None, emb_table=None, **_):
    """Full (unsharded) inputs in, full output out. mask_positions unused."""
    nc = _get_nc()
    in_maps = _shard_inputs(inputs, categories, emb_table)
    res = run_bass_kernel_spmd(nc, in_maps, list(range(N_CORES)))
    out = np.empty((B, S, D), dtype=np.float32)
    for i in range(N_CORES):
        yt = res.results[i]["yt"].reshape(P, N, DSUB)
        out[i * B_LOC:(i + 1) * B_LOC] = yt.transpose(1, 0, 2).reshape(
            N, D).astype(np.float32).reshape(B_LOC, S, D)
    return out


# revision 15
# speedup vs baseline: 3.2471x; 3.2471x over previous
"""Trainium2 Bass kernel for nn_CategoryMultiplier.

out[b, s, :] = inputs[b, s, :] * (emb_table[categories[b, s]] if
               categories[b, s] != 0 else 1.0)

Sharding: pure data parallel over batch. 8 cores x 16 batches each.
Precision: fp16 end-to-end (host converts f32->fp16 in, fp16->f32 out).

Gather strategy: NO gather at all. Every gather mechanism measured on
this stack is the bottleneck (HBM dma_gather serializes ~8ns/row on the
single SWDGE queue = 72us; ap_gather ucode ~30ns/row; transpose-mode
gather crashes the runtime). Instead, the host groups positions by
vocab tile (cat // 128) and the embedding lookup becomes a block-
diagonal one-hot matmul on the otherwise-idle PE:

  for each of 8 vocab tiles: stationary = one-hot.T block [128v, 128p]
  (built by one DVE is_equal against a per-partition iota), moving =
  the tile's 128 table rows [128v, 512f] resident in SBUF, psum out =
  [128 positions, 512] = exactly the gathered rows. ACT copies psum ->
  fp16, DVE multiplies by x, and the result DMAs out in sorted order;
  the host inverse-permutes. GpSimd does nothing (no ucode library
  load) and issues the y DMAs.

Per-tile position counts are padded to a fixed capacity C on the host
(pad positions have x=0). C=1152 covers the reference inputs (max 1123)
with margin; kernel() transparently rebuilds with a larger C if some
input ever exceeds it.

Padding (category 0 -> multiplier 1.0): host writes ones into table row
0 before uploading; one-hot then selects the ones row.
"""

import numpy as np

import concourse.bass as bass
import concourse.bacc as bacc
import concourse.mybir as mybir
import concourse.tile as tile
from concourse.bass_utils import run_bass_kernel_spmd

# Problem shape (hardcoded per harness contract).
B, S, D = 128, 512, 512
VOCAB = 1000
N_CORES = 8
B_LOC = B // N_CORES            # 16 batches per core
N = B_LOC * S                   # 8192 positions per core
P = 128                         # SBUF partitions
TILES = 8                       # vocab tiles of 128 rows
C_DEFAULT = 1152                # per-tile position capacity (blocks of 128)
SUP = 3                         # blocks per DMA super-chunk

F16 = mybir.dt.float16

_ALU = mybir.AluOpType


def _build_nc(cap):
    nblk = TILES * (cap // P)   # total 128-position blocks
    bpt = cap // P              # blocks per tile
    assert bpt % SUP == 0

    nc = bacc.Bacc("TRN2", target_bir_lowering=False, debug=False)

    xb = nc.dram_tensor("xb", [P, nblk * D], F16, kind="ExternalInput")
    catrep = nc.dram_tensor("catrep", [P, nblk * P], F16,
                            kind="ExternalInput")
    tabsb = nc.dram_tensor("tabsb", [P, TILES * D], F16,
                           kind="ExternalInput")
    yb = nc.dram_tensor("yb", [P, nblk * D], F16, kind="ExternalOutput")

    iota_dram = nc.inline_tensor(
        np.arange(P, dtype=np.float32).reshape(P, 1), name="iota_col")

    with tile.TileContext(nc) as tc:
        with (
            tc.tile_pool(name="const", bufs=1) as const_pool,
            tc.tile_pool(name="oh", bufs=2) as oh_pool,
            tc.tile_pool(name="io", bufs=4) as io_pool,
            tc.tile_pool(name="m", bufs=4) as m_pool,
            tc.psum_pool(name="ps", bufs=7) as ps_pool,
        ):
            iota_t = const_pool.tile([P, 1], mybir.dt.float32)
            nc.sync.dma_start(out=iota_t[:], in_=iota_dram[:])
            tab_t = const_pool.tile([P, TILES * D], F16)
            nc.sync.dma_start(out=tab_t[:], in_=tabsb[:])
            cat_t = const_pool.tile([P, nblk * P], F16)
            nc.sync.dma_start(out=cat_t[:], in_=catrep[:])

            for t in range(TILES):
                oh_t = oh_pool.tile([P, cap], F16, tag="oh")
                nc.vector.tensor_scalar(
                    out=oh_t[:], in0=cat_t[:, t * cap:(t + 1) * cap],
                    scalar1=iota_t[:, 0:1], scalar2=None, op0=_ALU.is_equal)

                for sc in range(bpt // SUP):
                    blk0 = t * bpt + sc * SUP
                    x_t = io_pool.tile([P, SUP * D], F16, tag="x")
                    nc.sync.dma_start(
                        out=x_t[:], in_=xb[:, blk0 * D:(blk0 + SUP) * D])
                    m_t = m_pool.tile([P, SUP * D], F16, tag="m")
                    for j in range(SUP):
                        lo = (sc * SUP + j) * P
                        ps_t = ps_pool.tile([P, D], mybir.dt.float32)
                        nc.tensor.matmul(
                            ps_t[:], oh_t[:, lo:lo + P],
                            tab_t[:, t * D:(t + 1) * D],
                            start=True, stop=True)
                        nc.scalar.copy(out=m_t[:, j * D:(j + 1) * D],
                                       in_=ps_t[:])
                    nc.vector.tensor_mul(out=m_t[:], in0=m_t[:], in1=x_t[:])
                    nc.gpsimd.dma_start(
                        out=yb[:, blk0 * D:(blk0 + SUP) * D], in_=m_t[:])

    nc.compile()
    return nc


_NC = {}


def _get_nc(cap=C_DEFAULT):
    if cap not in _NC:
        _NC[cap] = _build_nc(cap)
    return _NC[cap]


def _required_cap(categories):
    mx = 0
    for i in range(N_CORES):
        c = np.asarray(categories[i * B_LOC:(i + 1) * B_LOC]).reshape(N)
        mx = max(mx, int(np.bincount(c // P, minlength=TILES).max()))
    cap = C_DEFAULT
    while cap < mx:
        cap += P
    return cap


def _shard_inputs(inputs, categories, emb_table, cap=C_DEFAULT):
    nblk = TILES * (cap // P)
    npad = TILES * cap

    tab = np.zeros((TILES * P, D), dtype=np.float16)
    tab[:VOCAB] = np.asarray(emb_table).astype(np.float16)
    tab[0, :] = np.float16(1.0)
    # tabsb[k, t*D+f] = tab[t*128 + k, f]
    tab_sb = np.ascontiguousarray(
        tab.reshape(TILES, P, D).transpose(1, 0, 2)).reshape(P, TILES * D)

    in_maps = []
    perms = []
    for i in range(N_CORES):
        c = np.asarray(categories[i * B_LOC:(i + 1) * B_LOC]).reshape(N)
        c = c.astype(np.int32)
        tile_id = c >> 7
        order = np.argsort(tile_id, kind="stable")
        counts = np.bincount(tile_id, minlength=TILES)
        assert counts.max() <= cap, (counts.max(), cap)
        # slot layout: tile t occupies [t*cap, t*cap + counts[t])
        starts = np.zeros(TILES, np.int64)
        starts[1:] = np.cumsum(counts)[:-1]
        slot_of_sorted = (np.arange(N) - starts[tile_id[order]]
                          + cap * tile_id[order])
        perm = np.full(npad, -1, np.int64)
        perm[slot_of_sorted] = order          # slot -> original position

        x = np.asarray(inputs[i * B_LOC:(i + 1) * B_LOC]).astype(
            np.float16).reshape(N, D)
        x_pad = np.zeros((npad, D), np.float16)
        catrel = np.zeros(npad, np.float16)
        filled = perm >= 0
        x_pad[filled] = x[perm[filled]]
        catrel[filled] = (c[perm[filled]] & 127).astype(np.float16)

        # xb[p, blk*D+f] = x_pad[blk*128+p, f]
        xbv = np.ascontiguousarray(
            x_pad.reshape(nblk, P, D).transpose(1, 0, 2)).reshape(P, nblk * D)
        # catrep[k, blk*128+p] = catrel[blk*128+p]  (replicated over k)
        crep = np.ascontiguousarray(
            np.broadcast_to(catrel.reshape(1, npad), (P, npad)))
        in_maps.append({"xb": xbv, "catrep": crep, "tabsb": tab_sb})
        perms.append(perm)
    return in_maps, perms


def kernel(inputs, categories, mask_positions=None, emb_table=None, **_):
    """Full (unsharded) inputs in, full output out. mask_positions unused."""
    cap = _required_cap(categories)
    nc = _get_nc(cap)
    in_maps, perms = _shard_inputs(inputs, categories, emb_table, cap)
    res = run_bass_kernel_spmd(nc, in_maps, list(range(N_CORES)))
    nblk = TILES * (cap // P)
    npad = TILES * cap
    out = np.empty((B, S, D), dtype=np.float32)
    for i in range(N_CORES):
        # yb[p, blk*D+f] -> y_pad[blk*128+p, f] -> inverse permute
        yv = res.results[i]["yb"].reshape(P, nblk, D).transpose(1, 0, 2)
        yv = yv.reshape(npad, D)
        perm = perms[i]
        filled = perm >= 0
        y = np.empty((N, D), np.float32)
        y[perm[filled]] = yv[filled].astype(np.float32)
        out[i * B_LOC:(i + 1) * B_LOC] = y.reshape(B_LOC, S, D)
    return out


# revision 17
# speedup vs baseline: 3.3166x; 1.0214x over previous
"""Trainium2 Bass kernel for nn_CategoryMultiplier.

out[b, s, :] = inputs[b, s, :] * (emb_table[categories[b, s]] if
               categories[b, s] != 0 else 1.0)

Sharding: pure data parallel over batch. 8 cores x 16 batches each.
Precision: fp16 end-to-end (host converts f32->fp16 in, fp16->f32 out).

Gather strategy: NO gather at all. Every gather mechanism measured on
this stack is the bottleneck (HBM dma_gather serializes ~8ns/row on the
single SWDGE queue = 72us; ap_gather ucode ~30ns/row; transpose-mode
gather crashes the runtime). Instead, the host groups positions by
vocab tile (cat // 128) and the embedding lookup becomes a block-
diagonal one-hot matmul on the otherwise-idle PE:

  for each of 8 vocab tiles: stationary = one-hot.T block [128v, 128p]
  (built by one DVE is_equal against a per-partition iota), moving =
  the tile's 128 table rows [128v, 512f] resident in SBUF, psum out =
  [128 positions, 512] = exactly the gathered rows. ACT copies psum ->
  fp16, DVE multiplies by x, and the result DMAs out in sorted order;
  the host inverse-permutes. GpSimd does nothing (no ucode library
  load) and issues the y DMAs.

Per-tile position counts are padded to a fixed capacity C on the host
(pad positions have x=0). C=1152 covers the reference inputs (max 1123)
with margin; kernel() transparently rebuilds with a larger C if some
input ever exceeds it.

Padding (category 0 -> multiplier 1.0): host writes ones into table row
0 before uploading; one-hot then selects the ones row.
"""

import numpy as np

import concourse.bass as bass
import concourse.bacc as bacc
import concourse.mybir as mybir
import concourse.tile as tile
from concourse.bass_utils import run_bass_kernel_spmd

# Problem shape (hardcoded per harness contract).
B, S, D = 128, 512, 512
VOCAB = 1000
N_CORES = 8
B_LOC = B // N_CORES            # 16 batches per core
N = B_LOC * S                   # 8192 positions per core
P = 128                         # SBUF partitions
TILES = 8                       # vocab tiles of 128 rows
C_DEFAULT = 1152                # per-tile position capacity (blocks of 128)
SUP = 3                         # blocks per DMA super-chunk

F16 = mybir.dt.float16

_ALU = mybir.AluOpType


def _build_nc(cap):
    nblk = TILES * (cap // P)   # total 128-position blocks
    bpt = cap // P              # blocks per tile
    assert bpt % SUP == 0

    nc = bacc.Bacc("TRN2", target_bir_lowering=False, debug=False)

    xb = nc.dram_tensor("xb", [P, nblk * D], F16, kind="ExternalInput")
    catrep = nc.dram_tensor("catrep", [P, nblk * P], F16,
                            kind="ExternalInput")
    tabsb = nc.dram_tensor("tabsb", [P, TILES * D], F16,
                           kind="ExternalInput")
    yb = nc.dram_tensor("yb", [P, nblk * D], F16, kind="ExternalOutput")

    iota_dram = nc.inline_tensor(
        np.arange(P, dtype=np.float32).reshape(P, 1), name="iota_col")

    with tile.TileContext(nc) as tc:
        with (
            tc.tile_pool(name="const", bufs=1) as const_pool,
            tc.tile_pool(name="oh", bufs=2) as oh_pool,
            tc.tile_pool(name="io", bufs=4) as io_pool,
            tc.tile_pool(name="m", bufs=4) as m_pool,
            tc.psum_pool(name="ps", bufs=2) as ps_pool,
        ):
            iota_t = const_pool.tile([P, 1], mybir.dt.float32)
            nc.sync.dma_start(out=iota_t[:], in_=iota_dram[:])
            tab_t = const_pool.tile([P, TILES * D], F16)
            nc.sync.dma_start(out=tab_t[:], in_=tabsb[:])
            # Per-tile cat slices so tile 0's one-hot isn't gated on the
            # whole 2.4MB catrep transfer.
            cat_t = const_pool.tile([P, nblk * P], F16)
            for t in range(TILES):
                nc.scalar.dma_start(out=cat_t[:, t * cap:(t + 1) * cap],
                                    in_=catrep[:, t * cap:(t + 1) * cap])

            for t in range(TILES):
                oh_t = oh_pool.tile([P, cap], F16, tag="oh")
                nc.vector.tensor_scalar(
                    out=oh_t[:], in0=cat_t[:, t * cap:(t + 1) * cap],
                    scalar1=iota_t[:, 0:1], scalar2=None, op0=_ALU.is_equal)

                for sc in range(bpt // SUP):
                    blk0 = t * bpt + sc * SUP
                    x_t = io_pool.tile([P, SUP * D], F16, tag="x")
                    nc.sync.dma_start(
                        out=x_t[:], in_=xb[:, blk0 * D:(blk0 + SUP) * D])
                    m_t = m_pool.tile([P, SUP * D], F16, tag="m")
                    ps_t = ps_pool.tile([P, SUP * D], mybir.dt.float32)
                    for j in range(SUP):
                        lo = (sc * SUP + j) * P
                        nc.tensor.matmul(
                            ps_t[:, j * D:(j + 1) * D], oh_t[:, lo:lo + P],
                            tab_t[:, t * D:(t + 1) * D],
                            start=True, stop=True)
                    nc.scalar.copy(out=m_t[:], in_=ps_t[:])
                    nc.vector.tensor_mul(out=m_t[:], in0=m_t[:], in1=x_t[:])
                    nc.gpsimd.dma_start(
                        out=yb[:, blk0 * D:(blk0 + SUP) * D], in_=m_t[:])

    nc.compile()
    return nc


_NC = {}


def _get_nc(cap=C_DEFAULT):
    if cap not in _NC:
        _NC[cap] = _build_nc(cap)
    return _NC[cap]


def _required_cap(categories):
    mx = 0
    for i in range(N_CORES):
        c = np.asarray(categories[i * B_LOC:(i + 1) * B_LOC]).reshape(N)
        mx = max(mx, int(np.bincount(c // P, minlength=TILES).max()))
    cap = C_DEFAULT
    while cap < mx:
        cap += P
    return cap


def _shard_inputs(inputs, categories, emb_table, cap=C_DEFAULT):
    nblk = TILES * (cap // P)
    npad = TILES * cap

    tab = np.zeros((TILES * P, D), dtype=np.float16)
    tab[:VOCAB] = np.asarray(emb_table).astype(np.float16)
    tab[0, :] = np.float16(1.0)
    # tabsb[k, t*D+f] = tab[t*128 + k, f]
    tab_sb = np.ascontiguousarray(
        tab.reshape(TILES, P, D).transpose(1, 0, 2)).reshape(P, TILES * D)

    in_maps = []
    perms = []
    for i in range(N_CORES):
        c = np.asarray(categories[i * B_LOC:(i + 1) * B_LOC]).reshape(N)
        c = c.astype(np.int32)
        tile_id = c >> 7
        order = np.argsort(tile_id, kind="stable")
        counts = np.bincount(tile_id, minlength=TILES)
        assert counts.max() <= cap, (counts.max(), cap)
        # slot layout: tile t occupies [t*cap, t*cap + counts[t])
        starts = np.zeros(TILES, np.int64)
        starts[1:] = np.cumsum(counts)[:-1]
        slot_of_sorted = (np.arange(N) - starts[tile_id[order]]
                          + cap * tile_id[order])
        perm = np.full(npad, -1, np.int64)
        perm[slot_of_sorted] = order          # slot -> original position

        x = np.asarray(inputs[i * B_LOC:(i + 1) * B_LOC]).astype(
            np.float16).reshape(N, D)
        x_pad = np.zeros((npad, D), np.float16)
        catrel = np.zeros(npad, np.float16)
        filled = perm >= 0
        x_pad[filled] = x[perm[filled]]
        catrel[filled] = (c[perm[filled]] & 127).astype(np.float16)

        # xb[p, blk*D+f] = x_pad[blk*128+p, f]
        xbv = np.ascontiguousarray(
            x_pad.reshape(nblk, P, D).transpose(1, 0, 2)).reshape(P, nblk * D)
        # catrep[k, blk*128+p] = catrel[blk*128+p]  (replicated over k)
        crep = np.ascontiguousarray(
            np.broadcast_to(catrel.reshape(1, npad), (P, npad)))
        in_maps.append({"xb": xbv, "catrep": crep, "tabsb": tab_sb})
        perms.append(perm)
    return in_maps, perms


def kernel(inputs, categories, mask_positions=None, emb_table=None, **_):
    """Full (unsharded) inputs in, full output out. mask_positions unused."""
    cap = _required_cap(categories)
    nc = _get_nc(cap)
    in_maps, perms = _shard_inputs(inputs, categories, emb_table, cap)
    res = run_bass_kernel_spmd(nc, in_maps, list(range(N_CORES)))
    nblk = TILES * (cap // P)
    npad = TILES * cap
    out = np.empty((B, S, D), dtype=np.float32)
    for i in range(N_CORES):
        # yb[p, blk*D+f] -> y_pad[blk*128+p, f] -> inverse permute
        yv = res.results[i]["yb"].reshape(P, nblk, D).transpose(1, 0, 2)
        yv = yv.reshape(npad, D)
        perm = perms[i]
        filled = perm >= 0
        y = np.empty((N, D), np.float32)
        y[perm[filled]] = yv[filled].astype(np.float32)
        out[i * B_LOC:(i + 1) * B_LOC] = y.reshape(B_LOC, S, D)
    return out


# revision 19
# speedup vs baseline: 3.4638x; 1.0444x over previous
"""Trainium2 Bass kernel for nn_CategoryMultiplier.

out[b, s, :] = inputs[b, s, :] * (emb_table[categories[b, s]] if
               categories[b, s] != 0 else 1.0)

Sharding: pure data parallel over batch. 8 cores x 16 batches each.
Precision: fp16 end-to-end (host converts f32->fp16 in, fp16->f32 out).

Gather strategy: NO gather at all. Every gather mechanism measured on
this stack is the bottleneck (HBM dma_gather serializes ~8ns/row on the
single SWDGE queue = 72us; ap_gather ucode ~30ns/row; transpose-mode
gather crashes the runtime). Instead, the host groups positions by
vocab tile (cat // 128) and the embedding lookup becomes a block-
diagonal one-hot matmul on the otherwise-idle PE:

  for each of 8 vocab tiles: stationary = one-hot.T block [128v, 128p]
  (built by one DVE is_equal against a per-partition iota), moving =
  the tile's 128 table rows [128v, 512f] resident in SBUF, psum out =
  [128 positions, 512] = exactly the gathered rows. ACT copies psum ->
  fp16, DVE multiplies by x, and the result DMAs out in sorted order;
  the host inverse-permutes. GpSimd does nothing (no ucode library
  load) and issues the y DMAs.

Per-tile position counts are padded to a fixed capacity C on the host
(pad positions have x=0). C=1152 covers the reference inputs (max 1123)
with margin; kernel() transparently rebuilds with a larger C if some
input ever exceeds it.

Padding (category 0 -> multiplier 1.0): host writes ones into table row
0 before uploading; one-hot then selects the ones row.
"""

import numpy as np

import concourse.bass as bass
import concourse.bacc as bacc
import concourse.mybir as mybir
import concourse.tile as tile
from concourse.bass_utils import run_bass_kernel_spmd

# Problem shape (hardcoded per harness contract).
B, S, D = 128, 512, 512
VOCAB = 1000
N_CORES = 8
B_LOC = B // N_CORES            # 16 batches per core
N = B_LOC * S                   # 8192 positions per core
P = 128                         # SBUF partitions
TILES = 8                       # vocab tiles of 128 rows
C_DEFAULT = 1152                # per-tile position capacity (blocks of 128)
SUP = 3                         # blocks per DMA super-chunk

F16 = mybir.dt.float16

_ALU = mybir.AluOpType


def _build_nc(cap):
    nblk = TILES * (cap // P)   # total 128-position blocks
    bpt = cap // P              # blocks per tile
    assert bpt % SUP == 0

    nc = bacc.Bacc("TRN2", target_bir_lowering=False, debug=False)

    xb = nc.dram_tensor("xb", [P, nblk * D], F16, kind="ExternalInput")
    catrep = nc.dram_tensor("catrep", [P, nblk * P], F16,
                            kind="ExternalInput")
    tabsb = nc.dram_tensor("tabsb", [P, TILES * D], F16,
                           kind="ExternalInput")
    yb = nc.dram_tensor("yb", [P, nblk * D], F16, kind="ExternalOutput")

    iota_dram = nc.inline_tensor(
        np.arange(P, dtype=np.float32).reshape(P, 1), name="iota_col")

    with tile.TileContext(nc) as tc:
        with (
            tc.tile_pool(name="const", bufs=1) as const_pool,
            tc.tile_pool(name="oh", bufs=TILES) as oh_pool,
            tc.tile_pool(name="io", bufs=6) as io_pool,
            tc.tile_pool(name="m", bufs=6) as m_pool,
            tc.psum_pool(name="ps", bufs=2) as ps_pool,
        ):
            iota_t = const_pool.tile([P, 1], mybir.dt.float32)
            nc.sync.dma_start(out=iota_t[:], in_=iota_dram[:])
            tab_t = const_pool.tile([P, TILES * D], F16)
            nc.sync.dma_start(out=tab_t[:], in_=tabsb[:])
            # Separate per-tile cat tiles: slice-level DMA deps, so tile
            # t's one-hot only waits on its own 288KB transfer.
            cat_ts = []
            for t in range(TILES):
                ct = const_pool.tile([P, cap], F16, tag=f"cat{t}")
                nc.scalar.dma_start(out=ct[:],
                                    in_=catrep[:, t * cap:(t + 1) * cap])
                cat_ts.append(ct)

            # All one-hots generated up front so PE never stalls on DVE at
            # a tile boundary (DVE runs them before the first muls queue).
            oh_ts = []
            for t in range(TILES):
                oh_t = oh_pool.tile([P, cap], F16, tag="oh")
                nc.vector.tensor_scalar(
                    out=oh_t[:], in0=cat_ts[t][:],
                    scalar1=iota_t[:, 0:1], scalar2=None, op0=_ALU.is_equal)
                oh_ts.append(oh_t)

            for t in range(TILES):
                oh_t = oh_ts[t]
                for sc in range(bpt // SUP):
                    blk0 = t * bpt + sc * SUP
                    x_t = io_pool.tile([P, SUP * D], F16, tag="x")
                    nc.sync.dma_start(
                        out=x_t[:], in_=xb[:, blk0 * D:(blk0 + SUP) * D])
                    m_t = m_pool.tile([P, SUP * D], F16, tag="m")
                    ps_t = ps_pool.tile([P, SUP * D], mybir.dt.float32)
                    for j in range(SUP):
                        lo = (sc * SUP + j) * P
                        nc.tensor.matmul(
                            ps_t[:, j * D:(j + 1) * D], oh_t[:, lo:lo + P],
                            tab_t[:, t * D:(t + 1) * D],
                            start=True, stop=True)
                    nc.scalar.copy(out=m_t[:], in_=ps_t[:])
                    nc.vector.tensor_mul(out=m_t[:], in0=m_t[:], in1=x_t[:])
                    nc.gpsimd.dma_start(
                        out=yb[:, blk0 * D:(blk0 + SUP) * D], in_=m_t[:])

    nc.compile()
    return nc


_NC = {}


def _get_nc(cap=C_DEFAULT):
    if cap not in _NC:
        _NC[cap] = _build_nc(cap)
    return _NC[cap]


def _required_cap(categories):
    mx = 0
    for i in range(N_CORES):
        c = np.asarray(categories[i * B_LOC:(i + 1) * B_LOC]).reshape(N)
        mx = max(mx, int(np.bincount(c // P, minlength=TILES).max()))
    cap = C_DEFAULT
    while cap < mx:
        cap += P
    return cap


def _shard_inputs(inputs, categories, emb_table, cap=C_DEFAULT):
    nblk = TILES * (cap // P)
    npad = TILES * cap

    tab = np.zeros((TILES * P, D), dtype=np.float16)
    tab[:VOCAB] = np.asarray(emb_table).astype(np.float16)
    tab[0, :] = np.float16(1.0)
    # tabsb[k, t*D+f] = tab[t*128 + k, f]
    tab_sb = np.ascontiguousarray(
        tab.reshape(TILES, P, D).transpose(1, 0, 2)).reshape(P, TILES * D)

    in_maps = []
    perms = []
    for i in range(N_CORES):
        c = np.asarray(categories[i * B_LOC:(i + 1) * B_LOC]).reshape(N)
        c = c.astype(np.int32)
        tile_id = c >> 7
        order = np.argsort(tile_id, kind="stable")
        counts = np.bincount(tile_id, minlength=TILES)
        assert counts.max() <= cap, (counts.max(), cap)
        # slot layout: tile t occupies [t*cap, t*cap + counts[t])
        starts = np.zeros(TILES, np.int64)
        starts[1:] = np.cumsum(counts)[:-1]
        slot_of_sorted = (np.arange(N) - starts[tile_id[order]]
                          + cap * tile_id[order])
        perm = np.full(npad, -1, np.int64)
        perm[slot_of_sorted] = order          # slot -> original position

        x = np.asarray(inputs[i * B_LOC:(i + 1) * B_LOC]).astype(
            np.float16).reshape(N, D)
        x_pad = np.zeros((npad, D), np.float16)
        catrel = np.zeros(npad, np.float16)
        filled = perm >= 0
        x_pad[filled] = x[perm[filled]]
        catrel[filled] = (c[perm[filled]] & 127).astype(np.float16)

        # xb[p, blk*D+f] = x_pad[blk*128+p, f]
        xbv = np.ascontiguousarray(
            x_pad.reshape(nblk, P, D).transpose(1, 0, 2)).reshape(P, nblk * D)
        # catrep[k, blk*128+p] = catrel[blk*128+p]  (replicated over k)
        crep = np.ascontiguousarray(
            np.broadcast_to(catrel.reshape(1, npad), (P, npad)))
        in_maps.append({"xb": xbv, "catrep": crep, "tabsb": tab_sb})
        perms.append(perm)
    return in_maps, perms


def kernel(inputs, categories, mask_positions=None, emb_table=None, **_):
    """Full (unsharded) inputs in, full output out. mask_positions unused."""
    cap = _required_cap(categories)
    nc = _get_nc(cap)
    in_maps, perms = _shard_inputs(inputs, categories, emb_table, cap)
    res = run_bass_kernel_spmd(nc, in_maps, list(range(N_CORES)))
    nblk = TILES * (cap // P)
    npad = TILES * cap
    out = np.empty((B, S, D), dtype=np.float32)
    for i in range(N_CORES):
        # yb[p, blk*D+f] -> y_pad[blk*128+p, f] -> inverse permute
        yv = res.results[i]["yb"].reshape(P, nblk, D).transpose(1, 0, 2)
        yv = yv.reshape(npad, D)
        perm = perms[i]
        filled = perm >= 0
        y = np.empty((N, D), np.float32)
        y[perm[filled]] = yv[filled].astype(np.float32)
        out[i * B_LOC:(i + 1) * B_LOC] = y.reshape(B_LOC, S, D)
    return out


# revision 20
# speedup vs baseline: 3.8503x; 1.1116x over previous
"""Trainium2 Bass kernel for nn_CategoryMultiplier.

out[b, s, :] = inputs[b, s, :] * (emb_table[categories[b, s]] if
               categories[b, s] != 0 else 1.0)

Sharding: pure data parallel over batch. 8 cores x 16 batches each.
Precision: fp16 end-to-end (host converts f32->fp16 in, fp16->f32 out).

Gather strategy: NO gather at all. Every gather mechanism measured on
this stack bottlenecks (HBM dma_gather serializes ~8ns/row on the
single SWDGE queue = 72us; ap_gather ucode ~30ns/row; transpose-mode
gather crashes the runtime). Instead the embedding lookup runs as a
block-diagonal one-hot matmul on the otherwise-idle PE:

  The host partitions the vocab into 8 BINS of <=128 rows whose
  position counts are balanced to exactly N/8 = 1024 each (greedy +
  swap repair; falls back to contiguous bins + padding if an input
  can't be balanced). Positions are grouped by bin, so each 128-
  position block needs one matmul: stationary = one-hot.T [128v, 128p]
  (one DVE is_equal of the uint8 within-bin slot ids against a per-
  partition iota), moving = the bin's 128 table rows [128v, 512f]
  SBUF-resident, psum out = the gathered rows [128p, 512f]. ACT
  copies psum->fp16, DVE multiplies by x, GpSimd DMAs y out in sorted
  order, and the host inverse-permutes. Exact balance means zero x/y
  padding traffic.

Padding (category 0 -> multiplier 1.0): host writes ones into table
row 0; pad positions (fallback path only) carry slot id 255, which
matches no one-hot lane and yields y=0 (discarded).
"""

import numpy as np

import concourse.bass as bass
import concourse.bacc as bacc
import concourse.mybir as mybir
import concourse.tile as tile
from concourse.bass_utils import run_bass_kernel_spmd

# Problem shape (hardcoded per harness contract).
B, S, D = 128, 512, 512
VOCAB = 1000
N_CORES = 8
B_LOC = B // N_CORES            # 16 batches per core
N = B_LOC * S                   # 8192 positions per core
P = 128                         # SBUF partitions
TILES = 8                       # vocab bins of <=128 rows
CAP_BAL = N // TILES            # 1024: per-bin positions when balanced

F16 = mybir.dt.float16
U8 = mybir.dt.uint8

_ALU = mybir.AluOpType


def _build_nc(cap):
    bpt = cap // P              # blocks per bin
    nblk = TILES * bpt
    sup = 4 if bpt % 4 == 0 else 3   # blocks per super-chunk
    assert bpt % sup == 0

    nc = bacc.Bacc("TRN2", target_bir_lowering=False, debug=False)

    xb = nc.dram_tensor("xb", [P, nblk * D], F16, kind="ExternalInput")
    catrel = nc.dram_tensor("catrel", [P, nblk * P], U8,
                            kind="ExternalInput")
    tabsb = nc.dram_tensor("tabsb", [P, TILES * D], F16,
                           kind="ExternalInput")
    yb = nc.dram_tensor("yb", [P, nblk * D], F16, kind="ExternalOutput")

    iota_dram = nc.inline_tensor(
        np.arange(P, dtype=np.float32).reshape(P, 1), name="iota_col")

    with tile.TileContext(nc) as tc:
        with (
            tc.tile_pool(name="const", bufs=1) as const_pool,
            tc.tile_pool(name="oh", bufs=TILES) as oh_pool,
            tc.tile_pool(name="io", bufs=6) as io_pool,
            tc.tile_pool(name="m", bufs=6) as m_pool,
            tc.psum_pool(name="ps", bufs=8 // sup) as ps_pool,
        ):
            iota_t = const_pool.tile([P, 1], mybir.dt.float32)
            nc.sync.dma_start(out=iota_t[:], in_=iota_dram[:])
            tab_t = const_pool.tile([P, TILES * D], F16)
            nc.sync.dma_start(out=tab_t[:], in_=tabsb[:])
            # cat slices per bin on the (otherwise idle-at-start) gpsimd
            # queue so bin 0's one-hot unblocks after ~1 small DMA.
            cat_ts = []
            for t in range(TILES):
                ct = const_pool.tile([P, cap], U8, tag=f"cat{t}")
                nc.gpsimd.dma_start(out=ct[:],
                                    in_=catrel[:, t * cap:(t + 1) * cap])
                cat_ts.append(ct)

            # All one-hots up front: DVE finishes them before the muls
            # queue up, so PE never stalls at a bin boundary.
            oh_ts = []
            for t in range(TILES):
                oh_t = oh_pool.tile([P, cap], F16, tag="oh")
                nc.vector.tensor_scalar(
                    out=oh_t[:], in0=cat_ts[t][:],
                    scalar1=iota_t[:, 0:1], scalar2=None, op0=_ALU.is_equal)
                oh_ts.append(oh_t)

            for t in range(TILES):
                for sc in range(bpt // sup):
                    blk0 = t * bpt + sc * sup
                    x_t = io_pool.tile([P, sup * D], F16, tag="x")
                    nc.sync.dma_start(
                        out=x_t[:], in_=xb[:, blk0 * D:(blk0 + sup) * D])
                    m_t = m_pool.tile([P, sup * D], F16, tag="m")
                    ps_t = ps_pool.tile([P, sup * D], mybir.dt.float32)
                    for j in range(sup):
                        lo = (sc * sup + j) * P
                        nc.tensor.matmul(
                            ps_t[:, j * D:(j + 1) * D],
                            oh_ts[t][:, lo:lo + P],
                            tab_t[:, t * D:(t + 1) * D],
                            start=True, stop=True)
                    nc.scalar.copy(out=m_t[:], in_=ps_t[:])
                    nc.vector.tensor_mul(out=m_t[:], in0=m_t[:], in1=x_t[:])
                    nc.gpsimd.dma_start(
                        out=yb[:, blk0 * D:(blk0 + sup) * D], in_=m_t[:])

    nc.compile()
    return nc


_NC = {}


def _get_nc(cap=CAP_BAL):
    if cap not in _NC:
        _NC[cap] = _build_nc(cap)
    return _NC[cap]


def _balance_bins(counts):
    """Partition vocab rows into TILES bins of <=P rows with position
    counts summing exactly to CAP_BAL each. Returns row2bin or None."""
    order = np.argsort(-counts)
    bins = [[] for _ in range(TILES)]
    sums = np.zeros(TILES, np.int64)
    for v in order:
        for b in sorted(range(TILES), key=lambda b: (sums[b], len(bins[b]))):
            if len(bins[b]) < P:
                bins[b].append(int(v))
                sums[b] += counts[v]
                break
    for _ in range(20000):
        hi, lo = int(np.argmax(sums)), int(np.argmin(sums))
        if sums[hi] == CAP_BAL and sums[lo] == CAP_BAL:
            row2bin = np.empty(len(counts), np.int64)
            for b, rows in enumerate(bins):
                row2bin[rows] = b
            return row2bin
        diff = sums[hi] - CAP_BAL
        moved = False
        if len(bins[lo]) < P:
            best = None
            for v in bins[hi]:
                if 0 < counts[v] <= diff and (
                        best is None or counts[v] > counts[best]):
                    best = v
            if best is not None:
                bins[hi].remove(best)
                bins[lo].append(best)
                sums[hi] -= counts[best]
                sums[lo] += counts[best]
                moved = True
        if not moved:
            for a in bins[hi]:
                for bb in bins[lo]:
                    dd = counts[a] - counts[bb]
                    if 0 < dd <= diff:
                        bins[hi].remove(a)
                        bins[lo].remove(bb)
                        bins[hi].append(bb)
                        bins[lo].append(a)
                        sums[hi] -= dd
                        sums[lo] += dd
                        moved = True
                        break
                if moved:
                    break
        if not moved:
            return None
    return None


def _shard_inputs(inputs, categories, emb_table):
    """Returns (in_maps, perms, cap)."""
    tabf = np.zeros((TILES * P, D), dtype=np.float16)
    tabf[:VOCAB] = np.asarray(emb_table).astype(np.float16)
    tabf[0, :] = np.float16(1.0)

    cores = []
    cap = CAP_BAL
    for i in range(N_CORES):
        c = np.asarray(categories[i * B_LOC:(i + 1) * B_LOC]).reshape(N)
        c = c.astype(np.int64)
        counts = np.bincount(c, minlength=TILES * P)
        row2bin = _balance_bins(counts)
        if row2bin is None:                      # fallback: contiguous bins
            row2bin = np.arange(TILES * P) // P
            cap = max(cap, -(-int(np.bincount(
                c // P, minlength=TILES).max()) // P) * P)
        cores.append((c, row2bin))

    in_maps = []
    perms = []
    bpt = cap // P
    nblk = TILES * bpt
    npad = TILES * cap
    for c, row2bin in cores:
        # slot of each vocab row within its bin (stable order)
        row2slot = np.empty(TILES * P, np.int64)
        rows_of = []
        for b in range(TILES):
            rows = np.nonzero(row2bin == b)[0]
            row2slot[rows] = np.arange(len(rows))
            rows_of.append(rows)

        # per-core reordered table: tab_sb[k, b*D+f] = tabf[rows_of[b][k]]
        tab_sb = np.zeros((P, TILES * D), np.float16)
        for b in range(TILES):
            rows = rows_of[b]
            tab_sb[:len(rows), b * D:(b + 1) * D] = tabf[rows]

        bin_of_pos = row2bin[c]
        order = np.argsort(bin_of_pos, kind="stable")
        counts_b = np.bincount(bin_of_pos, minlength=TILES)
        assert counts_b.max() <= cap
        starts = np.zeros(TILES, np.int64)
        starts[1:] = np.cumsum(counts_b)[:-1]
        slot_of_sorted = (np.arange(N) - starts[bin_of_pos[order]]
                          + cap * bin_of_pos[order])
        perm = np.full(npad, -1, np.int64)
        perm[slot_of_sorted] = order

        x = np.asarray(inputs[len(perms) * B_LOC:
                              (len(perms) + 1) * B_LOC]).astype(
            np.float16).reshape(N, D)
        x_pad = np.zeros((npad, D), np.float16)
        crel = np.full(npad, 255, np.uint8)
        filled = perm >= 0
        x_pad[filled] = x[perm[filled]]
        crel[filled] = row2slot[c[perm[filled]]].astype(np.uint8)

        xbv = np.ascontiguousarray(
            x_pad.reshape(nblk, P, D).transpose(1, 0, 2)).reshape(P, nblk * D)
        crep = np.ascontiguousarray(
            np.broadcast_to(crel.reshape(1, npad), (P, npad)))
        in_maps.append({"xb": xbv, "catrel": crep, "tabsb": tab_sb})
        perms.append(perm)
    return in_maps, perms, cap


def kernel(inputs, categories, mask_positions=None, emb_table=None, **_):
    """Full (unsharded) inputs in, full output out. mask_positions unused."""
    in_maps, perms, cap = _shard_inputs(inputs, categories, emb_table)
    nc = _get_nc(cap)
    res = run_bass_kernel_spmd(nc, in_maps, list(range(N_CORES)))
    nblk = TILES * (cap // P)
    npad = TILES * cap
    out = np.empty((B, S, D), dtype=np.float32)
    for i in range(N_CORES):
        yv = res.results[i]["yb"].reshape(P, nblk, D).transpose(1, 0, 2)
        yv = yv.reshape(npad, D)
        perm = perms[i]
        filled = perm >= 0
        y = np.empty((N, D), np.float32)
        y[perm[filled]] = yv[filled].astype(np.float32)
        out[i * B_LOC:(i + 1) * B_LOC] = y.reshape(B_LOC, S, D)
    return out


# revision 21
# speedup vs baseline: 4.0349x; 1.0479x over previous
"""Trainium2 Bass kernel for nn_CategoryMultiplier.

out[b, s, :] = inputs[b, s, :] * (emb_table[categories[b, s]] if
               categories[b, s] != 0 else 1.0)

Sharding: pure data parallel over batch. 8 cores x 16 batches each.
Precision: fp16 end-to-end (host converts f32->fp16 in, fp16->f32 out).

Gather strategy: NO gather at all. Every gather mechanism measured on
this stack bottlenecks (HBM dma_gather serializes ~8ns/row on the
single SWDGE queue = 72us; ap_gather ucode ~30ns/row; transpose-mode
gather crashes the runtime). Instead the embedding lookup runs as a
block-diagonal one-hot matmul on the otherwise-idle PE:

  The host partitions the vocab into 8 BINS of <=128 rows whose
  position counts are balanced to exactly N/8 = 1024 each (greedy +
  swap repair; falls back to contiguous bins + padding if an input
  can't be balanced). Positions are grouped by bin, so each 128-
  position block needs one matmul: stationary = one-hot.T [128v, 128p]
  (one DVE is_equal of the uint8 within-bin slot ids against a per-
  partition iota), moving = the bin's 128 table rows [128v, 512f]
  SBUF-resident, psum out = the gathered rows [128p, 512f]. ACT
  copies psum->fp16, DVE multiplies by x, GpSimd DMAs y out in sorted
  order, and the host inverse-permutes. Exact balance means zero x/y
  padding traffic.

Padding (category 0 -> multiplier 1.0): host writes ones into table
row 0; pad positions (fallback path only) carry slot id 255, which
matches no one-hot lane and yields y=0 (discarded).
"""

import numpy as np

import concourse.bass as bass
import concourse.bacc as bacc
import concourse.mybir as mybir
import concourse.tile as tile
from concourse.bass_utils import run_bass_kernel_spmd

# Problem shape (hardcoded per harness contract).
B, S, D = 128, 512, 512
VOCAB = 1000
N_CORES = 8
B_LOC = B // N_CORES            # 16 batches per core
N = B_LOC * S                   # 8192 positions per core
P = 128                         # SBUF partitions
TILES = 8                       # vocab bins of <=128 rows
CAP_BAL = N // TILES            # 1024: per-bin positions when balanced

F16 = mybir.dt.float16
U8 = mybir.dt.uint8

_ALU = mybir.AluOpType


def _build_nc(cap):
    bpt = cap // P              # blocks per bin
    nblk = TILES * bpt
    sup = 4 if bpt % 4 == 0 else 3   # blocks per super-chunk
    assert bpt % sup == 0

    nc = bacc.Bacc("TRN2", target_bir_lowering=False, debug=False)

    xb = nc.dram_tensor("xb", [P, nblk * D], F16, kind="ExternalInput")
    catrel = nc.dram_tensor("catrel", [P, nblk * P], U8,
                            kind="ExternalInput")
    tabsb = nc.dram_tensor("tabsb", [P, TILES * D], F16,
                           kind="ExternalInput")
    yb = nc.dram_tensor("yb", [P, nblk * D], F16, kind="ExternalOutput")

    iota_dram = nc.inline_tensor(
        np.arange(P, dtype=np.float32).reshape(P, 1), name="iota_col")

    with tile.TileContext(nc) as tc:
        with (
            tc.tile_pool(name="const", bufs=1) as const_pool,
            tc.tile_pool(name="oh", bufs=TILES) as oh_pool,
            tc.tile_pool(name="io", bufs=6) as io_pool,
            tc.tile_pool(name="m", bufs=6) as m_pool,
            tc.psum_pool(name="ps", bufs=8 // sup) as ps_pool,
        ):
            iota_t = const_pool.tile([P, 1], mybir.dt.float32)
            nc.sync.dma_start(out=iota_t[:], in_=iota_dram[:])
            tab_t = const_pool.tile([P, TILES * D], F16)
            nc.sync.dma_start(out=tab_t[:], in_=tabsb[:])
            # cat slices per bin on the (otherwise idle-at-start) gpsimd
            # queue so bin 0's one-hot unblocks after ~1 small DMA.
            cat_ts = []
            for t in range(TILES):
                ct = const_pool.tile([P, cap], U8, tag=f"cat{t}")
                nc.gpsimd.dma_start(out=ct[:],
                                    in_=catrel[:, t * cap:(t + 1) * cap])
                cat_ts.append(ct)

            # All one-hots up front: DVE finishes them before the muls
            # queue up, so PE never stalls at a bin boundary.
            oh_ts = []
            for t in range(TILES):
                oh_t = oh_pool.tile([P, cap], F16, tag="oh")
                nc.vector.tensor_scalar(
                    out=oh_t[:], in0=cat_ts[t][:],
                    scalar1=iota_t[:, 0:1], scalar2=None, op0=_ALU.is_equal)
                oh_ts.append(oh_t)

            n_chunks = TILES * (bpt // sup)
            # A couple of chunks drain PSUM via a direct DVE multiply (f32
            # PSUM operand, 1x rate) instead of the ACT copy, so the two
            # engines drain concurrently and PE never waits on a bank.
            dve_drain = {n_chunks // 3, (2 * n_chunks) // 3}
            ci = 0
            for t in range(TILES):
                for sc in range(bpt // sup):
                    blk0 = t * bpt + sc * sup
                    x_t = io_pool.tile([P, sup * D], F16, tag="x")
                    nc.sync.dma_start(
                        out=x_t[:], in_=xb[:, blk0 * D:(blk0 + sup) * D])
                    m_t = m_pool.tile([P, sup * D], F16, tag="m")
                    ps_t = ps_pool.tile([P, sup * D], mybir.dt.float32)
                    for j in range(sup):
                        lo = (sc * sup + j) * P
                        nc.tensor.matmul(
                            ps_t[:, j * D:(j + 1) * D],
                            oh_ts[t][:, lo:lo + P],
                            tab_t[:, t * D:(t + 1) * D],
                            start=True, stop=True)
                    if ci in dve_drain:
                        nc.vector.tensor_mul(out=m_t[:], in0=x_t[:],
                                             in1=ps_t[:])
                    else:
                        nc.scalar.copy(out=m_t[:], in_=ps_t[:])
                        nc.vector.tensor_mul(out=m_t[:], in0=m_t[:],
                                             in1=x_t[:])
                    nc.gpsimd.dma_start(
                        out=yb[:, blk0 * D:(blk0 + sup) * D], in_=m_t[:])
                    ci += 1

    nc.compile()
    return nc


_NC = {}


def _get_nc(cap=CAP_BAL):
    if cap not in _NC:
        _NC[cap] = _build_nc(cap)
    return _NC[cap]


def _balance_bins(counts):
    """Partition vocab rows into TILES bins of <=P rows with position
    counts summing exactly to CAP_BAL each. Returns row2bin or None."""
    order = np.argsort(-counts)
    bins = [[] for _ in range(TILES)]
    sums = np.zeros(TILES, np.int64)
    for v in order:
        for b in sorted(range(TILES), key=lambda b: (sums[b], len(bins[b]))):
            if len(bins[b]) < P:
                bins[b].append(int(v))
                sums[b] += counts[v]
                break
    for _ in range(20000):
        hi, lo = int(np.argmax(sums)), int(np.argmin(sums))
        if sums[hi] == CAP_BAL and sums[lo] == CAP_BAL:
            row2bin = np.empty(len(counts), np.int64)
            for b, rows in enumerate(bins):
                row2bin[rows] = b
            return row2bin
        diff = sums[hi] - CAP_BAL
        moved = False
        if len(bins[lo]) < P:
            best = None
            for v in bins[hi]:
                if 0 < counts[v] <= diff and (
                        best is None or counts[v] > counts[best]):
                    best = v
            if best is not None:
                bins[hi].remove(best)
                bins[lo].append(best)
                sums[hi] -= counts[best]
                sums[lo] += counts[best]
                moved = True
        if not moved:
            for a in bins[hi]:
                for bb in bins[lo]:
                    dd = counts[a] - counts[bb]
                    if 0 < dd <= diff:
                        bins[hi].remove(a)
                        bins[lo].remove(bb)
                        bins[hi].append(bb)
                        bins[lo].append(a)
                        sums[hi] -= dd
                        sums[lo] += dd
                        moved = True
                        break
                if moved:
                    break
        if not moved:
            return None
    return None


def _shard_inputs(inputs, categories, emb_table):
    """Returns (in_maps, perms, cap)."""
    tabf = np.zeros((TILES * P, D), dtype=np.float16)
    tabf[:VOCAB] = np.asarray(emb_table).astype(np.float16)
    tabf[0, :] = np.float16(1.0)

    cores = []
    cap = CAP_BAL
    for i in range(N_CORES):
        c = np.asarray(categories[i * B_LOC:(i + 1) * B_LOC]).reshape(N)
        c = c.astype(np.int64)
        counts = np.bincount(c, minlength=TILES * P)
        row2bin = _balance_bins(counts)
        if row2bin is None:                      # fallback: contiguous bins
            row2bin = np.arange(TILES * P) // P
            cap = max(cap, -(-int(np.bincount(
                c // P, minlength=TILES).max()) // P) * P)
        cores.append((c, row2bin))

    in_maps = []
    perms = []
    bpt = cap // P
    nblk = TILES * bpt
    npad = TILES * cap
    for c, row2bin in cores:
        # slot of each vocab row within its bin (stable order)
        row2slot = np.empty(TILES * P, np.int64)
        rows_of = []
        for b in range(TILES):
            rows = np.nonzero(row2bin == b)[0]
            row2slot[rows] = np.arange(len(rows))
            rows_of.append(rows)

        # per-core reordered table: tab_sb[k, b*D+f] = tabf[rows_of[b][k]]
        tab_sb = np.zeros((P, TILES * D), np.float16)
        for b in range(TILES):
            rows = rows_of[b]
            tab_sb[:len(rows), b * D:(b + 1) * D] = tabf[rows]

        bin_of_pos = row2bin[c]
        order = np.argsort(bin_of_pos, kind="stable")
        counts_b = np.bincount(bin_of_pos, minlength=TILES)
        assert counts_b.max() <= cap
        starts = np.zeros(TILES, np.int64)
        starts[1:] = np.cumsum(counts_b)[:-1]
        slot_of_sorted = (np.arange(N) - starts[bin_of_pos[order]]
                          + cap * bin_of_pos[order])
        perm = np.full(npad, -1, np.int64)
        perm[slot_of_sorted] = order

        x = np.asarray(inputs[len(perms) * B_LOC:
                              (len(perms) + 1) * B_LOC]).astype(
            np.float16).reshape(N, D)
        x_pad = np.zeros((npad, D), np.float16)
        crel = np.full(npad, 255, np.uint8)
        filled = perm >= 0
        x_pad[filled] = x[perm[filled]]
        crel[filled] = row2slot[c[perm[filled]]].astype(np.uint8)

        xbv = np.ascontiguousarray(
            x_pad.reshape(nblk, P, D).transpose(1, 0, 2)).reshape(P, nblk * D)
        crep = np.ascontiguousarray(
            np.broadcast_to(crel.reshape(1, npad), (P, npad)))
        in_maps.append({"xb": xbv, "catrel": crep, "tabsb": tab_sb})
        perms.append(perm)
    return in_maps, perms, cap


def kernel(inputs, categories, mask_positions=None, emb_table=None, **_):
    """Full (unsharded) inputs in, full output out. mask_positions unused."""
    in_maps, perms, cap = _shard_inputs(inputs, categories, emb_table)
    nc = _get_nc(cap)
    res = run_bass_kernel_spmd(nc, in_maps, list(range(N_CORES)))
    nblk = TILES * (cap // P)
    npad = TILES * cap
    out = np.empty((B, S, D), dtype=np.float32)
    for i in range(N_CORES):
        yv = res.results[i]["yb"].reshape(P, nblk, D).transpose(1, 0, 2)
        yv = yv.reshape(npad, D)
        perm = perms[i]
        filled = perm >= 0
        y = np.empty((N, D), np.float32)
        y[perm[filled]] = yv[filled].astype(np.float32)
        out[i * B_LOC:(i + 1) * B_LOC] = y.reshape(B_LOC, S, D)
    return out
